# revision 7
# baseline (speedup 1.0000x reference)
"""Trainium2 Bass kernel for a single-layer multi-head attention block.

Reference computation (per batch element b):
    qkv = x @ w_qkv                       # [N, 3*768]
    q, k, v = split(qkv)                  # heads: 12 x 64
    scores = q @ k^T / sqrt(64)           # per head [N, N]
    attn = softmax(scores, axis=-1)
    out = attn @ v                        # [N, 768]
    y = out @ w_out + b_out               # [N, 768]

Sharding: batch (8) data-parallel across 8 NeuronCores, one batch element
per core. No collectives.

Per-core dataflow (all matmuls in float32r ~ tf32, rel err ~1.6e-4):
  1. xT = x^T via PE transposes                     [768, 1024]
  2. qkT = (x @ w_qk)^T = w_qk^T-free layout        [1536, 1024] (feat-major)
     v   = x @ w_v (natural, token-major)           [1024, 768] -> v_aug with
     a ones column per head ([keys, 65] per head slice)
  3. per head: scoresT = k_hT^T-free x q (keys-major) -> exp on ScalarE
     (scale=1/8 folded in).  AV: out_unnormT[65, q] = v_aug^T @ expT; row 64
     accumulates the softmax denominator (ones column trick).
  4. normalize: reciprocal of row 64, broadcast via DRAM bounce DMA,
     multiply rows 0..63 -> outT [768, 1024] (feat-major)
  5. y = outT^T @ w_out + b_out -> DMA out
"""

import numpy as np

import concourse.bacc as bacc
import concourse.mybir as mybir
import concourse.tile as tile
from concourse.bass_utils import run_bass_kernel_spmd
from concourse.masks import make_identity

N_CORES = 8
N = 1024          # tokens per batch element
E = 768           # embedding dim
H = 12            # heads
D = 64            # head dim
P = 128

f32 = mybir.dt.float32
f32r = mybir.dt.float32r
AF = mybir.ActivationFunctionType


def build_nc(n_heads=H, do_exp=True, do_av=True, do_final=True, do_qk=True):
    nc = bacc.Bacc("TRN2", target_bir_lowering=False, debug=False,
                   num_devices=N_CORES)

    x = nc.dram_tensor("x", [N, E], f32, kind="ExternalInput")
    w_qkv = nc.dram_tensor("w_qkv", [E, 3 * E], f32r, kind="ExternalInput")
    w_out = nc.dram_tensor("w_out", [E, E], f32r, kind="ExternalInput")
    b_out = nc.dram_tensor("b_out", [E], f32, kind="ExternalInput")
    ones_c = nc.dram_tensor("ones_const", [1], f32r, kind="ExternalInput")
    out = nc.dram_tensor("out", [N, E], f32, kind="ExternalOutput")
    inv_scratch = nc.dram_tensor("inv_scratch", [H, 2, 512], f32)

    with tile.TileContext(nc) as tc:
        with tc.tile_pool(name="pers", bufs=1) as pers, \
             tc.tile_pool(name="xnat", bufs=2) as xnat_pool, \
             tc.tile_pool(name="wqk", bufs=9) as wqk_pool, \
             tc.tile_pool(name="wv", bufs=6) as wv_pool, \
             tc.tile_pool(name="wout", bufs=7) as wout_pool, \
             tc.tile_pool(name="expp", bufs=4) as exp_pool, \
             tc.tile_pool(name="invp", bufs=2) as inv_pool, \
             tc.tile_pool(name="invbc", bufs=2) as invbc_pool, \
             tc.tile_pool(name="fin", bufs=2) as fin_pool, \
             tc.tile_pool(name="ps_small", bufs=4, space="PSUM") as ps_small, \
             tc.tile_pool(name="ps_big", bufs=2, space="PSUM") as ps_big:

            qkT = pers.tile([P, 12, N], f32r)        # [feat-chunk, j, tok]
            v_aug = pers.tile([P, 8, H, D + 1], f32r)  # [key-in-chunk, kc, h, d|1]
            outT = pers.tile([P, 6, N], f32r)
            xT = pers.tile([P, 6, N], f32r)
            b_bc = pers.tile([P, E], f32)
            ident = pers.tile([P, P], f32)

            # constants / preloads
            make_identity(nc, ident[:])
            nc.sync.dma_start(out=b_bc[:], in_=b_out[None, :].to_broadcast((P, E)))
            nc.sync.dma_start(
                out=v_aug[:].rearrange("p a h d -> p (a h) d")[:, :, D:D + 1],
                in_=ones_c[None, None, :].to_broadcast((P, 8 * H, 1)))

            # ---- phase 0: x -> xT (PE transpose) ----
            for tq in range(4):        # token pairs of tiles
                xns = []
                for t2 in range(2):
                    xn = xnat_pool.tile([P, E], f32, tag="xnat")
                    nc.sync.dma_start(
                        out=xn[:], in_=x[(tq * 2 + t2) * P:(tq * 2 + t2 + 1) * P, :])
                    xns.append(xn)
                for ec in range(6):
                    pt = ps_small.tile([P, 512], f32, tag="ps_small")
                    for t2 in range(2):
                        nc.tensor.transpose(
                            pt[:, t2 * P:(t2 + 1) * P],
                            xns[t2][:, ec * P:(ec + 1) * P], ident[:])
                    nc.vector.tensor_copy(
                        xT[:, ec, tq * 256:(tq + 1) * 256], pt[:, 0:256])

            # ---- phase 1b: v natural + ones augmentation ----
            for vf, fw in ((0, 512), (1, 256)):
                wvs = []
                for kc in range(6):
                    wv = wv_pool.tile([P, 512], f32r, tag="wv")
                    nc.sync.dma_start(
                        out=wv[:, 0:fw],
                        in_=w_qkv[kc * P:(kc + 1) * P,
                                  2 * E + vf * 512:2 * E + vf * 512 + fw])
                    wvs.append(wv)
                nheads = fw // D
                for t in range(8):
                    pv = ps_small.tile([P, 512], f32, tag="ps_small")
                    for kc in range(6):
                        nc.tensor.matmul(
                            pv[:, 0:fw],
                            xT[:, kc, t * P:(t + 1) * P],
                            wvs[kc][:, 0:fw],
                            start=(kc == 0), stop=(kc == 5))
                    nc.vector.tensor_copy(
                        v_aug[:, t, vf * 8:vf * 8 + nheads, 0:D],
                        pv[:].rearrange("p (h d) -> p h d", d=D)[:, 0:nheads, :])

            # ---- phase 1a: qkT (feature-major q and k) ----
            j_order = [0, 6, 1, 7, 2, 8, 3, 9, 4, 10, 5, 11]
            for j in (j_order if do_qk else []):
                wqs = []
                for kc in range(6):
                    wq = wqk_pool.tile([P, P], f32r, tag="wqk")
                    nc.sync.dma_start(
                        out=wq[:],
                        in_=w_qkv[kc * P:(kc + 1) * P, j * P:(j + 1) * P])
                    wqs.append(wq)
                for nt in range(2):
                    pq = ps_small.tile([P, 512], f32, tag="ps_small")
                    for kc in range(6):
                        nc.tensor.matmul(
                            pq[:],
                            wqs[kc][:],
                            xT[:, kc, nt * 512:(nt + 1) * 512],
                            start=(kc == 0), stop=(kc == 5))
                    nc.vector.tensor_copy(qkT[:, j, nt * 512:(nt + 1) * 512], pq[:])

            # ---- phase 2+3: per-head attention ----
            for h in range(n_heads):
                qp = (h % 2) * D          # partition base of head features
                jq = h // 2               # q tile index
                jk = 6 + h // 2           # k tile index
                exps = []
                for m in range(8):        # key tiles
                    ps = ps_big.tile([P, N], f32, tag="ps_big")
                    for nt in range(2):
                        nc.tensor.matmul(
                            ps[:, nt * 512:(nt + 1) * 512],
                            qkT[qp:qp + D, jk, m * P:(m + 1) * P],
                            qkT[qp:qp + D, jq, nt * 512:(nt + 1) * 512],
                            start=True, stop=True)
                    et = exp_pool.tile([P, N], f32r, tag="expp")
                    if do_exp:
                        nc.scalar.activation(et[:], ps[:], AF.Exp, scale=0.125)
                    else:
                        nc.vector.tensor_copy(et[:, 0:N:8], ps[:, 0:N:8])
                    exps.append(et)
                # AV with ones column: out rows 0..63 = V^T E, row 64 = sums
                if not do_av:
                    continue
                pavs = []
                for nt in range(2):
                    pav = ps_small.tile([P, 512], f32, tag="ps_small",
                                        name=f"pav_h{h}_n{nt}")
                    pavs.append(pav)
                for kc in range(8):
                    for nt in range(2):
                        nc.tensor.matmul(
                            pavs[nt][0:D + 1, :],
                            v_aug[:, kc, h, :],
                            exps[kc][:, nt * 512:(nt + 1) * 512],
                            start=(kc == 0), stop=(kc == 7))
                for nt in range(2):
                    inv = inv_pool.tile([D + 1, 512], f32, tag="invp")
                    nc.vector.reciprocal(inv[D:D + 1, :], pavs[nt][D:D + 1, :])
                    nc.sync.dma_start(out=inv_scratch[h, nt][None, :],
                                      in_=inv[D:D + 1, :])
                    ibc = invbc_pool.tile([D, 512], f32, tag="invbc")
                    nc.sync.dma_start(
                        out=ibc[:],
                        in_=inv_scratch[h, nt][None, :].to_broadcast((D, 512)))
                    nc.vector.tensor_mul(
                        outT[qp:qp + D, jq, nt * 512:(nt + 1) * 512],
                        pavs[nt][0:D, :], ibc[:])

            # ---- phase 4: output projection + bias ----
            for ft, (f0, fw) in enumerate(((0, 512), (512, 256)) if do_final else ()):
                wos = []
                for fc in range(6):
                    wo = wout_pool.tile([P, 512], f32r, tag="wout")
                    nc.sync.dma_start(
                        out=wo[:, 0:fw],
                        in_=w_out[fc * P:(fc + 1) * P, f0:f0 + fw])
                    wos.append(wo)
                for t in range(8):
                    pf = ps_small.tile([P, 512], f32, tag="ps_small")
                    for fc in range(6):
                        nc.tensor.matmul(
                            pf[:, 0:fw],
                            outT[:, fc, t * P:(t + 1) * P],
                            wos[fc][:, 0:fw],
                            start=(fc == 0), stop=(fc == 5))
                    fs = fin_pool.tile([P, 512], f32, tag="fin")
                    nc.vector.tensor_add(fs[:, 0:fw], pf[:, 0:fw],
                                         b_bc[:, f0:f0 + fw])
                    nc.sync.dma_start(out=out[t * P:(t + 1) * P, f0:f0 + fw],
                                      in_=fs[:, 0:fw])

    nc.compile()
    return nc


_NC = None


def _get_nc():
    global _NC
    if _NC is None:
        _NC = build_nc()
    return _NC


def kernel(x, w_qkv, w_out, b_out):
    nc = _get_nc()
    x = np.ascontiguousarray(np.asarray(x, dtype=np.float32))
    w_qkv = np.ascontiguousarray(np.asarray(w_qkv, dtype=np.float32))
    w_out = np.ascontiguousarray(np.asarray(w_out, dtype=np.float32))
    b_out = np.ascontiguousarray(np.asarray(b_out, dtype=np.float32))
    one = np.ones(1, dtype=np.float32)
    in_maps = [
        {"x": x[i], "w_qkv": w_qkv, "w_out": w_out, "b_out": b_out,
         "ones_const": one}
        for i in range(N_CORES)
    ]
    res = run_bass_kernel_spmd(nc, in_maps, core_ids=list(range(N_CORES)))
    return np.stack([res.results[i]["out"] for i in range(N_CORES)], axis=0)


# revision 9
# speedup vs baseline: 22.3547x; 22.3547x over previous
"""Trainium2 Bass kernel for a single-layer multi-head attention block.

Reference computation (per batch element b):
    qkv = x @ w_qkv                       # [N, 3*768]
    q, k, v = split(qkv)                  # heads: 12 x 64
    scores = q @ k^T / sqrt(64)           # per head [N, N]
    attn = softmax(scores, axis=-1)
    out = attn @ v                        # [N, 768]
    y = out @ w_out + b_out               # [N, 768]

Sharding: batch (8) data-parallel across 8 NeuronCores, one batch element
per core. No collectives.

Per-core dataflow (all matmuls in float32r ~ tf32, rel err ~1.6e-4):
  1. xT = x^T via PE transposes                     [768, 1024]
  2. qkT = (x @ w_qk)^T = w_qk^T-free layout        [1536, 1024] (feat-major)
     v   = x @ w_v (natural, token-major)           [1024, 768] -> v_aug with
     a ones column per head ([keys, 65] per head slice)
  3. per head: scoresT = k_hT^T-free x q (keys-major) -> exp on ScalarE
     (scale=1/8 folded in).  AV: out_unnormT[65, q] = v_aug^T @ expT; row 64
     accumulates the softmax denominator (ones column trick).
  4. normalize: reciprocal of row 64, broadcast via DRAM bounce DMA,
     multiply rows 0..63 -> outT [768, 1024] (feat-major)
  5. y = outT^T @ w_out + b_out -> DMA out
"""

import numpy as np

import concourse.bacc as bacc
import concourse.mybir as mybir
import concourse.tile as tile
from concourse.bass_utils import run_bass_kernel_spmd
from concourse.masks import make_identity

N_CORES = 8
N = 1024          # tokens per batch element
E = 768           # embedding dim
H = 12            # heads
D = 64            # head dim
P = 128

f32 = mybir.dt.float32
f32r = mybir.dt.float32r
AF = mybir.ActivationFunctionType


def build_nc(n_heads=H, do_exp=True, do_av=True, do_final=True, do_qk=True,
             reps=1):
    nc = bacc.Bacc("TRN2", target_bir_lowering=False, debug=False,
                   num_devices=N_CORES)

    x = nc.dram_tensor("x", [N, E], f32, kind="ExternalInput")
    w_qkv = nc.dram_tensor("w_qkv", [E, 3 * E], f32r, kind="ExternalInput")
    w_out = nc.dram_tensor("w_out", [E, E], f32r, kind="ExternalInput")
    b_out = nc.dram_tensor("b_out", [E], f32, kind="ExternalInput")
    ones_c = nc.dram_tensor("ones_const", [1], f32r, kind="ExternalInput")
    out = nc.dram_tensor("out", [N, E], f32, kind="ExternalOutput")
    inv_scratch = nc.dram_tensor("inv_scratch", [H, 2, 512], f32)

    with tile.TileContext(nc) as tc:
        with tc.tile_pool(name="pers", bufs=1) as pers, \
             tc.tile_pool(name="xnat", bufs=2) as xnat_pool, \
             tc.tile_pool(name="wqk", bufs=9) as wqk_pool, \
             tc.tile_pool(name="wv", bufs=6) as wv_pool, \
             tc.tile_pool(name="wout", bufs=7) as wout_pool, \
             tc.tile_pool(name="expp", bufs=4) as exp_pool, \
             tc.tile_pool(name="invp", bufs=2) as inv_pool, \
             tc.tile_pool(name="invbc", bufs=2) as invbc_pool, \
             tc.tile_pool(name="fin", bufs=2) as fin_pool, \
             tc.tile_pool(name="ps_small", bufs=4, space="PSUM") as ps_small, \
             tc.tile_pool(name="ps_big", bufs=2, space="PSUM") as ps_big:

          for _rep in range(reps):
            qkT = pers.tile([P, 12, N], f32r, tag="qkT")        # [feat-chunk, j, tok]
            v_aug = pers.tile([P, 8, H, D + 1], f32r)  # [key-in-chunk, kc, h, d|1]
            outT = pers.tile([P, 6, N], f32r)
            xT = pers.tile([P, 6, N], f32r)
            b_bc = pers.tile([P, E], f32)
            ident = pers.tile([P, P], f32)

            # constants / preloads
            make_identity(nc, ident[:])
            nc.sync.dma_start(out=b_bc[:], in_=b_out[None, :].to_broadcast((P, E)))
            nc.sync.dma_start(
                out=v_aug[:].rearrange("p a h d -> p (a h) d")[:, :, D:D + 1],
                in_=ones_c[None, None, :].to_broadcast((P, 8 * H, 1)))

            # ---- phase 0: x -> xT (PE transpose) ----
            for tq in range(4):        # token pairs of tiles
                xns = []
                for t2 in range(2):
                    xn = xnat_pool.tile([P, E], f32, tag="xnat")
                    nc.sync.dma_start(
                        out=xn[:], in_=x[(tq * 2 + t2) * P:(tq * 2 + t2 + 1) * P, :])
                    xns.append(xn)
                for ec in range(6):
                    pt = ps_small.tile([P, 512], f32, tag="ps_small")
                    for t2 in range(2):
                        nc.tensor.transpose(
                            pt[:, t2 * P:(t2 + 1) * P],
                            xns[t2][:, ec * P:(ec + 1) * P], ident[:])
                    nc.vector.tensor_copy(
                        xT[:, ec, tq * 256:(tq + 1) * 256], pt[:, 0:256])

            # ---- phase 1b: v natural + ones augmentation ----
            for vf, fw in ((0, 512), (1, 256)):
                wvs = []
                for kc in range(6):
                    wv = wv_pool.tile([P, 512], f32r, tag="wv")
                    nc.sync.dma_start(
                        out=wv[:, 0:fw],
                        in_=w_qkv[kc * P:(kc + 1) * P,
                                  2 * E + vf * 512:2 * E + vf * 512 + fw])
                    wvs.append(wv)
                nheads = fw // D
                for t in range(8):
                    pv = ps_small.tile([P, 512], f32, tag="ps_small")
                    for kc in range(6):
                        nc.tensor.matmul(
                            pv[:, 0:fw],
                            xT[:, kc, t * P:(t + 1) * P],
                            wvs[kc][:, 0:fw],
                            start=(kc == 0), stop=(kc == 5))
                    nc.vector.tensor_copy(
                        v_aug[:, t, vf * 8:vf * 8 + nheads, 0:D],
                        pv[:].rearrange("p (h d) -> p h d", d=D)[:, 0:nheads, :])

            # ---- phase 1a: qkT (feature-major q and k) ----
            j_order = [0, 6, 1, 7, 2, 8, 3, 9, 4, 10, 5, 11]
            for j in (j_order if do_qk else []):
                wqs = []
                for kc in range(6):
                    wq = wqk_pool.tile([P, P], f32r, tag="wqk")
                    nc.sync.dma_start(
                        out=wq[:],
                        in_=w_qkv[kc * P:(kc + 1) * P, j * P:(j + 1) * P])
                    wqs.append(wq)
                for nt in range(2):
                    pq = ps_small.tile([P, 512], f32, tag="ps_small")
                    for kc in range(6):
                        nc.tensor.matmul(
                            pq[:],
                            wqs[kc][:],
                            xT[:, kc, nt * 512:(nt + 1) * 512],
                            start=(kc == 0), stop=(kc == 5))
                    nc.vector.tensor_copy(qkT[:, j, nt * 512:(nt + 1) * 512], pq[:])

            # ---- phase 2+3: per-head attention ----
            for h in range(n_heads):
                qp = (h % 2) * D          # partition base of head features
                jq = h // 2               # q tile index
                jk = 6 + h // 2           # k tile index
                exps = []
                for m in range(8):        # key tiles
                    ps = ps_big.tile([P, N], f32, tag="ps_big")
                    for nt in range(2):
                        nc.tensor.matmul(
                            ps[:, nt * 512:(nt + 1) * 512],
                            qkT[qp:qp + D, jk, m * P:(m + 1) * P],
                            qkT[qp:qp + D, jq, nt * 512:(nt + 1) * 512],
                            start=True, stop=True)
                    et = exp_pool.tile([P, N], f32r, tag="expp")
                    if do_exp:
                        nc.scalar.activation(et[:], ps[:], AF.Exp, scale=0.125)
                    else:
                        nc.vector.tensor_copy(et[:, 0:N:8], ps[:, 0:N:8])
                    exps.append(et)
                # AV with ones column: out rows 0..63 = V^T E, row 64 = sums
                if not do_av:
                    continue
                pavs = []
                for nt in range(2):
                    pav = ps_small.tile([P, 512], f32, tag="ps_small",
                                        name=f"pav_h{h}_n{nt}")
                    pavs.append(pav)
                for kc in range(8):
                    for nt in range(2):
                        nc.tensor.matmul(
                            pavs[nt][0:D + 1, :],
                            v_aug[:, kc, h, :],
                            exps[kc][:, nt * 512:(nt + 1) * 512],
                            start=(kc == 0), stop=(kc == 7))
                for nt in range(2):
                    inv = inv_pool.tile([D + 1, 512], f32, tag="invp")
                    nc.vector.reciprocal(inv[D:D + 1, :], pavs[nt][D:D + 1, :])
                    nc.sync.dma_start(out=inv_scratch[h, nt][None, :],
                                      in_=inv[D:D + 1, :])
                    ibc = invbc_pool.tile([D, 512], f32, tag="invbc")
                    nc.sync.dma_start(
                        out=ibc[:],
                        in_=inv_scratch[h, nt][None, :].to_broadcast((D, 512)))
                    nc.vector.tensor_mul(
                        outT[qp:qp + D, jq, nt * 512:(nt + 1) * 512],
                        pavs[nt][0:D, :], ibc[:])

            # ---- phase 4: output projection + bias ----
            for ft, (f0, fw) in enumerate(((0, 512), (512, 256)) if do_final else ()):
                wos = []
                for fc in range(6):
                    wo = wout_pool.tile([P, 512], f32r, tag="wout")
                    nc.sync.dma_start(
                        out=wo[:, 0:fw],
                        in_=w_out[fc * P:(fc + 1) * P, f0:f0 + fw])
                    wos.append(wo)
                for t in range(8):
                    pf = ps_small.tile([P, 512], f32, tag="ps_small")
                    for fc in range(6):
                        nc.tensor.matmul(
                            pf[:, 0:fw],
                            outT[:, fc, t * P:(t + 1) * P],
                            wos[fc][:, 0:fw],
                            start=(fc == 0), stop=(fc == 5))
                    fs = fin_pool.tile([P, 512], f32, tag="fin")
                    nc.vector.tensor_add(fs[:, 0:fw], pf[:, 0:fw],
                                         b_bc[:, f0:f0 + fw])
                    nc.sync.dma_start(out=out[t * P:(t + 1) * P, f0:f0 + fw],
                                      in_=fs[:, 0:fw])

    nc.compile()
    return nc


_NC = None


def _get_nc():
    global _NC
    if _NC is None:
        _NC = build_nc()
    return _NC


def kernel(x, w_qkv, w_out, b_out):
    nc = _get_nc()
    x = np.ascontiguousarray(np.asarray(x, dtype=np.float32))
    w_qkv = np.ascontiguousarray(np.asarray(w_qkv, dtype=np.float32))
    w_out = np.ascontiguousarray(np.asarray(w_out, dtype=np.float32))
    b_out = np.ascontiguousarray(np.asarray(b_out, dtype=np.float32))
    one = np.ones(1, dtype=np.float32)
    in_maps = [
        {"x": x[i], "w_qkv": w_qkv, "w_out": w_out, "b_out": b_out,
         "ones_const": one}
        for i in range(N_CORES)
    ]
    res = run_bass_kernel_spmd(nc, in_maps, core_ids=list(range(N_CORES)))
    return np.stack([res.results[i]["out"] for i in range(N_CORES)], axis=0)


# revision 10
# speedup vs baseline: 36.3966x; 1.6281x over previous
"""Trainium2 Bass kernel for a single-layer multi-head attention block.

Reference computation (per batch element):
    qkv = x @ w_qkv; q,k,v = split(qkv); 12 heads x 64
    out_h = softmax(q_h k_h^T / 8) v_h;  y = concat(out) @ w_out + b_out

Sharding: batch (8) data-parallel across 8 NeuronCores, one element/core.

The execution backend charges a roughly size-independent cost per
instruction, so the kernel is organized to minimize instruction count:
  - xT loaded directly with transposed (strided) DMA reads, no PE transposes
  - weights loaded with 6 wide DMAs per matrix
  - scores for 2 key-tiles packed into one [128,2048] PSUM tile -> one exp
  - softmax denominators via a ones-column appended to V (row 64 of the AV
    matmul output), normalized with one reciprocal + DRAM-bounce broadcast
    DMA + one multiply per head
  - all matmuls in float32r (tf32-like, ~1.6e-4 per-matmul rel err)
"""

import numpy as np

import concourse.bacc as bacc
import concourse.mybir as mybir
import concourse.tile as tile
from concourse.bass_utils import run_bass_kernel_spmd

N_CORES = 8
N = 1024          # tokens per batch element
E = 768           # embedding dim
H = 12            # heads
D = 64            # head dim
P = 128

f32 = mybir.dt.float32
f32r = mybir.dt.float32r
AF = mybir.ActivationFunctionType


def build_nc(n_heads=H, do_final=True, do_qk=True, reps=1):
    nc = bacc.Bacc("TRN2", target_bir_lowering=False, debug=False,
                   num_devices=N_CORES)

    x = nc.dram_tensor("x", [N, E], f32r, kind="ExternalInput")
    w_qkv = nc.dram_tensor("w_qkv", [E, 3 * E], f32r, kind="ExternalInput")
    w_out = nc.dram_tensor("w_out", [E, E], f32r, kind="ExternalInput")
    b_out = nc.dram_tensor("b_out", [E], f32, kind="ExternalInput")
    ones_c = nc.dram_tensor("ones_const", [1], f32r, kind="ExternalInput")
    out = nc.dram_tensor("out", [N, E], f32, kind="ExternalOutput")
    inv_scratch = nc.dram_tensor("inv_scratch", [H, N], f32)

    with tile.TileContext(nc) as tc:
      with tc.tile_pool(name="pers", bufs=1) as pers, \
           tc.tile_pool(name="ps1", bufs=2, space="PSUM") as ps1, \
           tc.tile_pool(name="ps2", bufs=1, space="PSUM") as ps2p:

        for _rep in range(reps):
            qkT = pers.tile([P, 12, N], f32r, tag="qkT")   # feat-major q|k
            v_aug = pers.tile([P, 8, H, D + 1], f32r, tag="v_aug")
            outT = pers.tile([P, 6, N], f32r, tag="outT")
            xT = pers.tile([P, 6, N], f32r, tag="xT")
            b_bc = pers.tile([P, E], f32, tag="b_bc")

            nc.sync.dma_start(out=b_bc[:], in_=b_out[None, :].to_broadcast((P, E)))
            nc.sync.dma_start(
                out=v_aug[:].rearrange("p a h d -> p (a h) d")[:, :, D:D + 1],
                in_=ones_c[None, None, :].to_broadcast((P, 8 * H, 1)))

            # ---- xT via transposed strided DMA reads ----
            xt_src = x.transpose([1, 0])       # [E, N] view of DRAM
            for ec in range(6):
                nc.sync.dma_start(out=xT[:, ec, :],
                                  in_=xt_src[ec * P:(ec + 1) * P, :])

            # ---- phase 1: qkT (feat-major) and v (token-major, augmented) ----
            with tc.tile_pool(name="wq", bufs=6) as wq_pool:
                wqs = []
                for kc in range(6):
                    wq = wq_pool.tile([P, 2 * E], f32r, tag="wq",
                                      name=f"wq_{kc}")
                    nc.sync.dma_start(out=wq[:],
                                      in_=w_qkv[kc * P:(kc + 1) * P, 0:2 * E])
                    wqs.append(wq)
                for j in (range(12) if do_qk else []):
                    pq = ps1.tile([P, N], f32, tag="ps1", name=f"pq_{j}")
                    for nt in range(2):
                        for kc in range(6):
                            nc.tensor.matmul(
                                pq[:, nt * 512:(nt + 1) * 512],
                                wqs[kc][:, j * P:(j + 1) * P],
                                xT[:, kc, nt * 512:(nt + 1) * 512],
                                start=(kc == 0), stop=(kc == 5))
                    nc.vector.tensor_copy(qkT[:, j, :], pq[:])

            with tc.tile_pool(name="wv", bufs=6) as wv_pool:
                wvs = []
                for kc in range(6):
                    wv = wv_pool.tile([P, E], f32r, tag="wv", name=f"wv_{kc}")
                    nc.sync.dma_start(out=wv[:],
                                      in_=w_qkv[kc * P:(kc + 1) * P, 2 * E:3 * E])
                    wvs.append(wv)
                for t in range(8):
                    pv = ps1.tile([P, N], f32, tag="ps1", name=f"pv_{t}")
                    for vf, f0, fw in ((0, 0, 512), (1, 512, 256)):
                        for kc in range(6):
                            nc.tensor.matmul(
                                pv[:, f0:f0 + fw],
                                xT[:, kc, t * P:(t + 1) * P],
                                wvs[kc][:, f0:f0 + fw],
                                start=(kc == 0), stop=(kc == 5))
                    nc.vector.tensor_copy(
                        v_aug[:, t, :, 0:D],
                        pv[:, 0:E].rearrange("p (h d) -> p h d", d=D))

            # ---- attention per head ----
            with tc.tile_pool(name="expp", bufs=3) as exp_pool, \
                 tc.tile_pool(name="invp", bufs=2) as inv_pool, \
                 tc.tile_pool(name="ibcp", bufs=2) as ibc_pool:
                for h in range(n_heads):
                    qp = (h % 2) * D
                    jq = h // 2
                    jk = 6 + h // 2
                    exps = []
                    for mq in range(4):
                        ps2 = ps2p.tile([P, 2 * N], f32, tag="ps2",
                                        name=f"ps2_{h}_{mq}")
                        for s2 in range(2):
                            m = 2 * mq + s2
                            for nt in range(2):
                                nc.tensor.matmul(
                                    ps2[:, s2 * N + nt * 512:
                                        s2 * N + (nt + 1) * 512],
                                    qkT[qp:qp + D, jk, m * P:(m + 1) * P],
                                    qkT[qp:qp + D, jq, nt * 512:(nt + 1) * 512],
                                    start=True, stop=True)
                        et = exp_pool.tile([P, 2 * N], f32r, tag="expp",
                                           name=f"exp_{h}_{mq}")
                        nc.scalar.activation(et[:], ps2[:], AF.Exp, scale=0.125)
                        exps.append(et)
                    pav = ps1.tile([P, N], f32, tag="ps1", name=f"pav_{h}")
                    for kc in range(8):
                        for nt in range(2):
                            nc.tensor.matmul(
                                pav[0:D + 1, nt * 512:(nt + 1) * 512],
                                v_aug[:, kc, h, :],
                                exps[kc // 2][:, (kc % 2) * N + nt * 512:
                                              (kc % 2) * N + (nt + 1) * 512],
                                start=(kc == 0), stop=(kc == 7))
                    inv = inv_pool.tile([D + 1, N], f32, tag="invp",
                                        name=f"inv_{h}")
                    nc.vector.reciprocal(inv[D:D + 1, :], pav[D:D + 1, :])
                    nc.sync.dma_start(out=inv_scratch[h][None, :],
                                      in_=inv[D:D + 1, :])
                    ibc = ibc_pool.tile([D, N], f32, tag="ibcp",
                                        name=f"ibc_{h}")
                    nc.sync.dma_start(
                        out=ibc[:],
                        in_=inv_scratch[h][None, :].to_broadcast((D, N)))
                    nc.vector.tensor_mul(outT[qp:qp + D, jq, :],
                                         pav[0:D, :], ibc[:])

            # ---- output projection + bias ----
            with tc.tile_pool(name="wout", bufs=6) as wout_pool, \
                 tc.tile_pool(name="finp", bufs=2) as fin_pool:
                wos = []
                for fc in range(6):
                    wo = wout_pool.tile([P, E], f32r, tag="wout",
                                        name=f"wo_{fc}")
                    nc.sync.dma_start(out=wo[:],
                                      in_=w_out[fc * P:(fc + 1) * P, :])
                    wos.append(wo)
                for t in (range(8) if do_final else []):
                    pf = ps1.tile([P, N], f32, tag="ps1", name=f"pf_{t}")
                    for ft, f0, fw in ((0, 0, 512), (1, 512, 256)):
                        for fc in range(6):
                            nc.tensor.matmul(
                                pf[:, f0:f0 + fw],
                                outT[:, fc, t * P:(t + 1) * P],
                                wos[fc][:, f0:f0 + fw],
                                start=(fc == 0), stop=(fc == 5))
                    fs = fin_pool.tile([P, E], f32, tag="fin", name=f"fs_{t}")
                    nc.vector.tensor_add(fs[:], pf[:, 0:E], b_bc[:])
                    nc.sync.dma_start(out=out[t * P:(t + 1) * P, :], in_=fs[:])

    nc.compile()
    return nc


_NC = None


def _get_nc():
    global _NC
    if _NC is None:
        _NC = build_nc()
    return _NC


def kernel(x, w_qkv, w_out, b_out):
    nc = _get_nc()
    x = np.ascontiguousarray(np.asarray(x, dtype=np.float32))
    w_qkv = np.ascontiguousarray(np.asarray(w_qkv, dtype=np.float32))
    w_out = np.ascontiguousarray(np.asarray(w_out, dtype=np.float32))
    b_out = np.ascontiguousarray(np.asarray(b_out, dtype=np.float32))
    one = np.ones(1, dtype=np.float32)
    in_maps = [
        {"x": x[i], "w_qkv": w_qkv, "w_out": w_out, "b_out": b_out,
         "ones_const": one}
        for i in range(N_CORES)
    ]
    res = run_bass_kernel_spmd(nc, in_maps, core_ids=list(range(N_CORES)))
    return np.stack([res.results[i]["out"] for i in range(N_CORES)], axis=0)
